# revision 5
# baseline (speedup 1.0000x reference)
"""Multi-head causal attention (B=4, T=2048, D=1024, H=16, HD=64) on 8 TRN2 cores.

Sharding: core = 2*b + g  (b in 0..3 batch, g in 0..1 head-group of 8 heads;
tensor-parallel on the QKV output columns / Wo rows).
Each core computes, for its (b, g):
  QT,KT = Wq_g^T x_b^T + b    layout [512, T] (d on partitions)
  V     = x_b Wv_g + bv       layout [T, 512] (t on partitions), with a ones
                              column appended per head (for softmax colsums)
  per head h: S^T = K_h Q_h^T (scale 1/8), E = exp(S^T) causal-masked,
  AV matmul gives unnormalized ctx^T [64, tq] + colsums row; normalize, then
  partial out = ctx @ Wo_g (+ bo on g==0 cores), shape [T, 1024].
Host sums the two partials per batch element.

Matmuls run in float32r (tf32-like; ~1.5e-4 rel err) at full PE rate.
"""
import numpy as np
from contextlib import ExitStack

import concourse.bacc as bacc
import concourse.bass as bass
import concourse.mybir as mybir
import concourse.tile as tile

F32 = mybir.dt.float32
F32R = mybir.dt.float32r
AF = mybir.ActivationFunctionType

B, T, DIN, DOUT, H = 4, 2048, 1024, 1024, 16
DL = 512          # local d_out slice (8 heads)
NH = 8            # local heads
S = 512           # tq strip width
NS = T // S       # 4 strips
KC = DIN // 128   # 8 k-chunks for projections
CC = DL // 128    # 4 dlocal chunks (head pairs)
NT = T // 128     # 16 tk tiles
VW = NH * 65      # V' width: 8 heads x (64 + ones col)


def _build_nc():
    nc = bacc.Bacc("TRN2", target_bir_lowering=False, debug=False,
                   enable_asserts=False)
    xT_d = nc.dram_tensor("xT", [DIN, T], F32, kind="ExternalInput").ap()
    wq_d = nc.dram_tensor("wq", [DIN, DL], F32, kind="ExternalInput").ap()
    wk_d = nc.dram_tensor("wk", [DIN, DL], F32, kind="ExternalInput").ap()
    wv_d = nc.dram_tensor("wv", [DIN, DL], F32, kind="ExternalInput").ap()
    bq_d = nc.dram_tensor("bq", [DL, 1], F32, kind="ExternalInput").ap()
    bk_d = nc.dram_tensor("bk", [DL, 1], F32, kind="ExternalInput").ap()
    bv_d = nc.dram_tensor("bv", [1, DL], F32, kind="ExternalInput").ap()
    wo_d = nc.dram_tensor("wo", [DL, DOUT], F32, kind="ExternalInput").ap()
    bo_d = nc.dram_tensor("bo", [1, DOUT], F32, kind="ExternalInput").ap()
    tri_d = nc.dram_tensor("tri", [128, 128], F32, kind="ExternalInput").ap()
    out_d = nc.dram_tensor("out", [T, DOUT], F32, kind="ExternalOutput").ap()

    with tile.TileContext(nc) as tc, ExitStack() as ctx:
        const = ctx.enter_context(tc.tile_pool(name="const", bufs=1))
        qtp = ctx.enter_context(tc.tile_pool(name="qtp", bufs=1))
        vpp = ctx.enter_context(tc.tile_pool(name="vpp", bufs=1))
        ctxp = ctx.enter_context(tc.tile_pool(name="ctxp", bufs=1))
        # PSUM banks: pa 2x[128,512]=2; sp 2x[128,1024]=4; cp 2x[65,512]=2
        pa = ctx.enter_context(tc.tile_pool(name="pa", bufs=2, space="PSUM"))
        sp = ctx.enter_context(tc.tile_pool(name="sp", bufs=2, space="PSUM"))
        cp = ctx.enter_context(tc.tile_pool(name="cp", bufs=1, space="PSUM"))

        # ---- constants ----
        onecols_f = const.tile([128, NH], F32)
        nc.vector.memset(onecols_f[:], 1.0)
        tri_t = const.tile([128, 128], F32)
        nc.sync.dma_start(tri_t[:], tri_d[:])
        bq_t = const.tile([128, CC], F32)
        nc.sync.dma_start(bq_t[:], bq_d.rearrange("(c p) o -> p (c o)", p=128))
        bk_t = const.tile([128, CC], F32)
        nc.sync.dma_start(bk_t[:], bk_d.rearrange("(c p) o -> p (c o)", p=128))


        # ---- persistent tensors ----
        qt = [qtp.tile([128, T], F32R, name=f"qt{c}") for c in range(CC)]
        kt = [qtp.tile([128, T], F32R, name=f"kt{c}") for c in range(CC)]
        vp = [vpp.tile([128, VW], F32R, name=f"vp{j}") for j in range(NT)]
        ctxt = [ctxp.tile([128, T], F32R, name=f"ctxt{c}") for c in range(CC)]

        # ones column of V' (col 64 of each 65-wide head block)
        for j in range(NT):
            nc.vector.tensor_copy(
                vp[j].rearrange("p (h w) -> p h w", w=65)[:, :, 64:65],
                onecols_f.rearrange("p (h o) -> p h o", o=1))

        # =============== Phase A: projections ===============
        with tc.tile_pool(name="wts", bufs=1) as wts, \
             tc.tile_pool(name="stage", bufs=2) as stage, \
             tc.tile_pool(name="xsp", bufs=1) as xsp:
            bv_f = wts.tile([1, DL], F32)
            nc.sync.dma_start(bv_f[:], bv_d[:])
            bvb = wts.tile([128, DL], F32)
            nc.gpsimd.partition_broadcast(bvb[:], bv_f[:])

            def load_w(dram, nm):
                ts = []
                for k in range(KC):
                    st = stage.tile([128, DL], F32, tag="stg", name="st")
                    nc.sync.dma_start(st[:], dram[k * 128:(k + 1) * 128, :])
                    t = wts.tile([128, DL], F32R, name=f"{nm}{k}")
                    nc.vector.tensor_copy(t[:], st[:])
                    ts.append(t)
                return ts

            wq_t = load_w(wq_d, "wqt")
            wk_t = load_w(wk_d, "wkt")
            wv_t = load_w(wv_d, "wvt")

            for s in range(NS):
                xs = []
                for k in range(KC):
                    st = stage.tile([128, S], F32, tag="stg", name="st")
                    nc.sync.dma_start(
                        st[:], xT_d[k * 128:(k + 1) * 128, s * S:(s + 1) * S])
                    xr = xsp.tile([128, S], F32R, tag=f"xs{k}", name="xr")
                    nc.vector.tensor_copy(xr[:], st[:])
                    xs.append(xr)
                for c in range(CC):
                    pq = pa.tile([128, S], F32, tag="ps", name="pq")
                    for k in range(KC):
                        nc.tensor.matmul(
                            pq[:], wq_t[k][:, c * 128:(c + 1) * 128], xs[k][:],
                            start=(k == 0), stop=(k == KC - 1))
                    nc.scalar.activation(qt[c][:, s * S:(s + 1) * S], pq[:],
                                         AF.Identity, bias=bq_t[:, c:c + 1])
                    pk = pa.tile([128, S], F32, tag="ps", name="pk")
                    for k in range(KC):
                        nc.tensor.matmul(
                            pk[:], wk_t[k][:, c * 128:(c + 1) * 128], xs[k][:],
                            start=(k == 0), stop=(k == KC - 1))
                    nc.scalar.activation(kt[c][:, s * S:(s + 1) * S], pk[:],
                                         AF.Identity, bias=bk_t[:, c:c + 1])
                for m in range(4):
                    pv = pa.tile([128, DL], F32, tag="ps", name="pv")
                    for k in range(KC):
                        nc.tensor.matmul(
                            pv[:], xs[k][:, m * 128:(m + 1) * 128], wv_t[k][:],
                            start=(k == 0), stop=(k == KC - 1))
                    j = s * 4 + m
                    nc.vector.tensor_add(
                        vp[j].rearrange("p (h w) -> p h w", w=65)[:, :, 0:64],
                        pv.rearrange("p (h w) -> p h w", w=64),
                        bvb.rearrange("p (h w) -> p h w", w=64))

        # =============== Phase B: attention ===============
        # Diagonal-region packing per (strip q4, head): the 4 partial blocks
        # j = 4*q4 + db cover tq columns [128*db, 512) with width w = 512-128*db;
        # they are packed into two PSUM tiles to amortize ACT overhead:
        #   tile1: db0 at cols 0:512, db1 at 512:896 ; tile2: db2 at 0:256, db3 at 256:384
        DIAG = [[(0, 0, 512), (1, 512, 384)], [(2, 0, 256), (3, 256, 128)]]
        with tc.tile_pool(name="ep", bufs=2) as ep, \
             tc.tile_pool(name="rp", bufs=2) as rp:
            for q4 in range(NS):
                q_sl = slice(q4 * S, (q4 + 1) * S)
                for c in range(CC):
                    cps = [cp.tile([65, S], F32, tag="cA", name="cA"),
                           cp.tile([65, S], F32, tag="cB", name="cB")]
                    nf = 4 * q4
                    # --- full blocks, two per PSUM tile ---
                    for grp in range(nf // 2):
                        for hi, boff in enumerate((0, 64)):
                            h = 2 * c + hi
                            st = sp.tile([128, 1024], F32, tag="s", name="sS")
                            for i in range(2):
                                kb = 2 * grp + i
                                nc.tensor.matmul(
                                    st[:, i * 512:(i + 1) * 512],
                                    kt[c][boff:boff + 64, kb * 128:(kb + 1) * 128],
                                    qt[c][boff:boff + 64, q_sl],
                                    start=True, stop=True,
                                    tile_position=(boff, 0))
                            et = ep.tile([128, 1024], F32R, tag="e", name="eS")
                            nc.scalar.activation(et[:], st[:], AF.Exp, scale=0.125)
                            for i in range(2):
                                kb = 2 * grp + i
                                nc.tensor.matmul(
                                    cps[hi][:], vp[kb][:, h * 65:h * 65 + 65],
                                    et[:, i * 512:(i + 1) * 512],
                                    start=(kb == 0), stop=False)
                    # --- diagonal region ---
                    for hi, boff in enumerate((0, 64)):
                        h = 2 * c + hi
                        sts = [sp.tile([128, 1024], F32, tag="s", name="sD1"),
                               sp.tile([128, 1024], F32, tag="s", name="sD2")]
                        ets = []
                        for stt, blks in zip(sts, DIAG):
                            for db, off, w in blks:
                                j = nf + db
                                nc.tensor.matmul(
                                    stt[:, off:off + w],
                                    kt[c][boff:boff + 64, j * 128:(j + 1) * 128],
                                    qt[c][boff:boff + 64,
                                          q4 * S + 128 * db:q4 * S + 128 * db + w],
                                    start=True, stop=True,
                                    tile_position=(boff, 0))
                            tot = sum(w for _, _, w in blks)
                            et = ep.tile([128, 1024], F32R, tag="e", name="eD")
                            nc.scalar.activation(et[:, 0:tot], stt[:, 0:tot],
                                                 AF.Exp, scale=0.125)
                            for db, off, w in blks:
                                nc.vector.tensor_mul(et[:, off:off + 128],
                                                     et[:, off:off + 128],
                                                     tri_t[:])
                            ets.append(et)
                        for et, blks in zip(ets, DIAG):
                            for db, off, w in blks:
                                j = nf + db
                                nc.tensor.matmul(
                                    cps[hi][:, S - w:S],
                                    vp[j][:, h * 65:h * 65 + 65],
                                    et[:, off:off + w],
                                    start=(j == 0), stop=(db == 3))
                    # --- normalize both heads ---
                    for hi in range(2):
                        rec = rp.tile([1, S], F32, tag="rec", name="rec")
                        nc.vector.reciprocal(rec[:], cps[hi][64:65, :])
                        rbc = rp.tile([64, S], F32, tag="rbc", name="rbc")
                        nc.gpsimd.partition_broadcast(rbc[:], rec[:])
                        nc.vector.tensor_mul(
                            ctxt[c][hi * 64:hi * 64 + 64, q_sl],
                            cps[hi][0:64, :], rbc[:])

        # =============== Phase C: output projection ===============
        with tc.tile_pool(name="wop", bufs=1) as wop, \
             tc.tile_pool(name="stg2", bufs=2) as stg2, \
             tc.tile_pool(name="op", bufs=3) as op:
            bo_f = wop.tile([1, DOUT], F32)
            nc.sync.dma_start(bo_f[:], bo_d[:])
            bob = wop.tile([128, DOUT], F32)
            nc.gpsimd.partition_broadcast(bob[:], bo_f[:])
            wo_t = []
            for c in range(CC):
                st = stg2.tile([128, DOUT], F32, tag="st2", name="st2")
                nc.sync.dma_start(st[:], wo_d[c * 128:(c + 1) * 128, :])
                t = wop.tile([128, DOUT], F32R, name=f"wot{c}")
                nc.vector.tensor_copy(t[:], st[:])
                wo_t.append(t)
            for m in range(NT):
                for n in range(2):
                    po = pa.tile([128, 512], F32, tag="ps", name="po")
                    for c in range(CC):
                        nc.tensor.matmul(
                            po[:], ctxt[c][:, m * 128:(m + 1) * 128],
                            wo_t[c][:, n * 512:(n + 1) * 512],
                            start=(c == 0), stop=(c == CC - 1))
                    ot = op.tile([128, 512], F32, tag="ot", name="ot")
                    nc.vector.tensor_add(ot[:], po[:],
                                         bob[:, n * 512:(n + 1) * 512])
                    nc.sync.dma_start(
                        out_d[m * 128:(m + 1) * 128, n * 512:(n + 1) * 512],
                        ot[:])

    nc.compile()
    return nc


_NC = None


def _get_nc():
    global _NC
    if _NC is None:
        _NC = _build_nc()
    return _NC


def make_in_maps(x, w_q, b_q, w_k, b_k, w_v, b_v, w_o, b_o):
    tri = np.triu(np.ones((128, 128), dtype=np.float32))
    in_maps = []
    for core in range(8):
        b, g = core // 2, core % 2
        sl = slice(g * DL, (g + 1) * DL)
        bo = b_o if g == 0 else np.zeros_like(b_o)
        in_maps.append({
            "xT": np.ascontiguousarray(x[b].T).astype(np.float32),
            "wq": np.ascontiguousarray(w_q[:, sl]).astype(np.float32),
            "wk": np.ascontiguousarray(w_k[:, sl]).astype(np.float32),
            "wv": np.ascontiguousarray(w_v[:, sl]).astype(np.float32),
            "bq": np.ascontiguousarray(b_q[sl].reshape(DL, 1)).astype(np.float32),
            "bk": np.ascontiguousarray(b_k[sl].reshape(DL, 1)).astype(np.float32),
            "bv": np.ascontiguousarray(b_v[sl].reshape(1, DL)).astype(np.float32),
            "wo": np.ascontiguousarray(w_o[sl, :]).astype(np.float32),
            "bo": np.ascontiguousarray(bo.reshape(1, DOUT)).astype(np.float32),
            "tri": tri,
        })
    return in_maps


def kernel(x, w_q, b_q, w_k, b_k, w_v, b_v, w_o, b_o):
    from concourse.bass_utils import run_bass_kernel_spmd
    nc = _get_nc()
    in_maps = make_in_maps(np.asarray(x), np.asarray(w_q), np.asarray(b_q),
                           np.asarray(w_k), np.asarray(b_k), np.asarray(w_v),
                           np.asarray(b_v), np.asarray(w_o), np.asarray(b_o))
    res = run_bass_kernel_spmd(nc, in_maps, core_ids=list(range(8)))
    outs = [r["out"] for r in res.results]
    full = np.stack([outs[2 * b] + outs[2 * b + 1] for b in range(B)])
    return full.astype(np.float32)


# revision 13
# speedup vs baseline: 174.4971x; 174.4971x over previous
"""Multi-head causal attention (B=4, T=2048, D=1024, H=16, HD=64) on 8 TRN2 cores.

Sharding: core = 2*b + g  (b in 0..3 batch, g in 0..1 head-group of 8 heads;
tensor-parallel on the QKV output columns / Wo rows).
Each core computes, for its (b, g):
  QT,KT = Wq_g^T x_b^T + b    layout [512, T] (d on partitions)
  V     = x_b Wv_g + bv       layout [T, 512] (t on partitions), with a ones
                              column appended per head (for softmax colsums)
  per head h: S^T = K_h Q_h^T (scale 1/8), E = exp(S^T) causal-masked,
  AV matmul gives unnormalized ctx^T [64, tq] + colsums row; normalize, then
  partial out = ctx @ Wo_g (+ bo on g==0 cores), shape [T, 1024].
Host sums the two partials per batch element.

Projection (strip s) and attention (query strip s) are interleaved so the
ACT-heavy softmax work of strip s overlaps the PE-dense projections of
strip s+1. Matmuls run in float32r (tf32-like; ~1.9e-4 rel err end-to-end).
"""
import numpy as np
from contextlib import ExitStack

import concourse.bacc as bacc
import concourse.bass as bass
import concourse.mybir as mybir
import concourse.tile as tile

F32 = mybir.dt.float32
F32R = mybir.dt.float32r
AF = mybir.ActivationFunctionType

B, T, DIN, DOUT, H = 4, 2048, 1024, 1024, 16
DL = 512          # local d_out slice (8 heads)
NH = 8            # local heads
S = 512           # tq strip width
NS = T // S       # 4 strips
KC = DIN // 128   # 8 k-chunks for projections
CC = DL // 128    # 4 dlocal chunks (head pairs)
NT = T // 128     # 16 tk tiles
VW = NH * 65      # V' width: 8 heads x (64 + ones col)

# Diagonal-region packing per (strip, head): the 4 partial blocks
# j = 4*s + db cover strip-local tq columns [128*db, 512), width w = 512-128*db.
# Packed into two PSUM tiles to amortize ACT instruction overhead:
#   tile1: db0 at cols 0:512, db1 at 512:896 ; tile2: db2 at 0:256, db3 at 256:384
DIAG = [[(0, 0, 512), (1, 512, 384)], [(2, 0, 256), (3, 256, 128)]]


def _build_nc(reps=1):
    nc = bacc.Bacc("TRN2", target_bir_lowering=False, debug=False,
                   enable_asserts=False)
    xT_d = nc.dram_tensor("xT", [DIN, T], F32R, kind="ExternalInput").ap()
    wq_d = nc.dram_tensor("wq", [DIN, DL], F32R, kind="ExternalInput").ap()
    wk_d = nc.dram_tensor("wk", [DIN, DL], F32R, kind="ExternalInput").ap()
    wv_d = nc.dram_tensor("wv", [DIN, DL], F32R, kind="ExternalInput").ap()
    bq_d = nc.dram_tensor("bq", [DL, 1], F32, kind="ExternalInput").ap()
    bk_d = nc.dram_tensor("bk", [DL, 1], F32, kind="ExternalInput").ap()
    bv_d = nc.dram_tensor("bv", [1, DL], F32, kind="ExternalInput").ap()
    wo_d = nc.dram_tensor("wo", [DL, DOUT], F32R, kind="ExternalInput").ap()
    bo_d = nc.dram_tensor("bo", [1, DOUT], F32, kind="ExternalInput").ap()
    tri_d = nc.dram_tensor("tri", [128, 128], F32, kind="ExternalInput").ap()
    out_d = nc.dram_tensor("out", [T, DOUT], F32, kind="ExternalOutput").ap()

    with tile.TileContext(nc) as tc:
      for _rep in range(reps):
        with ExitStack() as ctx:
          const = ctx.enter_context(tc.tile_pool(name="const", bufs=1))
          ktp = ctx.enter_context(tc.tile_pool(name="ktp", bufs=1))
          vpp = ctx.enter_context(tc.tile_pool(name="vpp", bufs=1))
          ctxp = ctx.enter_context(tc.tile_pool(name="ctxp", bufs=1))
          # PSUM banks: pp 2x[128,1024]=4; cp 2x2x[65,512]=4
          pp = ctx.enter_context(tc.tile_pool(name="pp", bufs=2, space="PSUM"))
          cp = ctx.enter_context(tc.tile_pool(name="cp", bufs=2, space="PSUM"))

          # ---- constants ----
          onecols_f = const.tile([128, NH], F32)
          nc.vector.memset(onecols_f[:], 1.0)
          tri_t = const.tile([128, 128], F32)
          nc.sync.dma_start(tri_t[:], tri_d[:])
          bq_t = const.tile([128, CC], F32)
          nc.sync.dma_start(bq_t[:], bq_d.rearrange("(c p) o -> p (c o)", p=128))
          bk_t = const.tile([128, CC], F32)
          nc.sync.dma_start(bk_t[:], bk_d.rearrange("(c p) o -> p (c o)", p=128))

          # ---- persistent tensors ----
          kt = [ktp.tile([128, T], F32R, name=f"kt{c}") for c in range(CC)]
          vp = [vpp.tile([128, VW], F32R, name=f"vp{j}") for j in range(NT)]
          ctxt = [ctxp.tile([128, T], F32R, name=f"ctxt{c}") for c in range(CC)]

          # ones column of V' (col 64 of each 65-wide head block)
          for j in range(NT):
              nc.vector.tensor_copy(
                  vp[j].rearrange("p (h w) -> p h w", w=65)[:, :, 64:65],
                  onecols_f.rearrange("p (h o) -> p h o", o=1))

          # ========= interleaved projections + attention, per strip =========
          with tc.tile_pool(name="wts", bufs=1) as wts, \
               tc.tile_pool(name="xsp", bufs=1) as xsp, \
               tc.tile_pool(name="qsp", bufs=2) as qsp, \
               tc.tile_pool(name="ep", bufs=2) as ep, \
               tc.tile_pool(name="rp", bufs=2) as rp:
              bv_f = wts.tile([1, DL], F32)
              nc.sync.dma_start(bv_f[:], bv_d[:])
              bvb = wts.tile([128, DL], F32)
              nc.gpsimd.partition_broadcast(bvb[:], bv_f[:])

              def load_w(dram, nm):
                  ts = []
                  for k in range(KC):
                      t = wts.tile([128, DL], F32R, name=f"{nm}{k}")
                      nc.sync.dma_start(t[:], dram[k * 128:(k + 1) * 128, :])
                      ts.append(t)
                  return ts

              wq_t = load_w(wq_d, "wqt")
              wk_t = load_w(wk_d, "wkt")
              wv_t = load_w(wv_d, "wvt")

              for s in range(NS):
                  # ---- projections for strip s ----
                  xs = []
                  for k in range(KC):
                      xr = xsp.tile([128, S], F32R, tag=f"xs{k}", name="xr",
                                    bufs=(2 if k < 5 else 1))
                      nc.sync.dma_start(
                          xr[:], xT_d[k * 128:(k + 1) * 128, s * S:(s + 1) * S])
                      xs.append(xr)
                  qt = []
                  for c in range(CC):
                      pq = pp.tile([128, S], F32, tag="s", name="pq")
                      for k in range(KC):
                          nc.tensor.matmul(
                              pq[:], wq_t[k][:, c * 128:(c + 1) * 128], xs[k][:],
                              start=(k == 0), stop=(k == KC - 1))
                      qs = qsp.tile([128, S], F32R, tag=f"qt{c}", name="qs")
                      nc.scalar.activation(qs[:], pq[:],
                                           AF.Identity, bias=bq_t[:, c:c + 1])
                      qt.append(qs)
                      pk = pp.tile([128, S], F32, tag="s", name="pk")
                      for k in range(KC):
                          nc.tensor.matmul(
                              pk[:], wk_t[k][:, c * 128:(c + 1) * 128], xs[k][:],
                              start=(k == 0), stop=(k == KC - 1))
                      nc.scalar.activation(kt[c][:, s * S:(s + 1) * S], pk[:],
                                           AF.Identity, bias=bk_t[:, c:c + 1])
                  for m in range(4):
                      pv = pp.tile([128, DL], F32, tag="s", name="pv")
                      for k in range(KC):
                          nc.tensor.matmul(
                              pv[:], xs[k][:, m * 128:(m + 1) * 128], wv_t[k][:],
                              start=(k == 0), stop=(k == KC - 1))
                      j = s * 4 + m
                      nc.vector.tensor_add(
                          vp[j].rearrange("p (h w) -> p h w", w=65)[:, :, 0:64],
                          pv.rearrange("p (h w) -> p h w", w=64),
                          bvb.rearrange("p (h w) -> p h w", w=64))

                  # ---- attention for query strip q4 = s ----
                  q_sl = slice(s * S, (s + 1) * S)
                  nf = 4 * s
                  for c in range(CC):
                      cps = [cp.tile([65, S], F32, tag="cA", name="cA"),
                             cp.tile([65, S], F32, tag="cB", name="cB")]
                      # full blocks, two per PSUM tile
                      for grp in range(nf // 2):
                          for hi, boff in enumerate((0, 64)):
                              h = 2 * c + hi
                              st = pp.tile([128, 1024], F32, tag="s", name="sS")
                              for i in range(2):
                                  kb = 2 * grp + i
                                  nc.tensor.matmul(
                                      st[:, i * 512:(i + 1) * 512],
                                      kt[c][boff:boff + 64,
                                            kb * 128:(kb + 1) * 128],
                                      qt[c][boff:boff + 64, :],
                                      start=True, stop=True,
                                      tile_position=(boff, 0))
                              et = ep.tile([128, 1024], F32R, tag="e", name="eS")
                              nc.scalar.activation(et[:], st[:], AF.Exp,
                                                   scale=0.125)
                              for i in range(2):
                                  kb = 2 * grp + i
                                  nc.tensor.matmul(
                                      cps[hi][:], vp[kb][:, h * 65:h * 65 + 65],
                                      et[:, i * 512:(i + 1) * 512],
                                      start=(kb == 0), stop=False)
                      # diagonal region
                      for hi, boff in enumerate((0, 64)):
                          h = 2 * c + hi
                          ets = []
                          for blks in DIAG:
                              stt = pp.tile([128, 1024], F32, tag="s", name="sD")
                              for db, off, w in blks:
                                  j = nf + db
                                  nc.tensor.matmul(
                                      stt[:, off:off + w],
                                      kt[c][boff:boff + 64,
                                            j * 128:(j + 1) * 128],
                                      qt[c][boff:boff + 64,
                                            128 * db:128 * db + w],
                                      start=True, stop=True,
                                      tile_position=(boff, 0))
                              tot = sum(w for _, _, w in blks)
                              et = ep.tile([128, 1024], F32R, tag="e", name="eD")
                              nc.scalar.activation(et[:, 0:tot], stt[:, 0:tot],
                                                   AF.Exp, scale=0.125)
                              for db, off, w in blks:
                                  nc.vector.tensor_mul(et[:, off:off + 128],
                                                       et[:, off:off + 128],
                                                       tri_t[:])
                              ets.append(et)
                          for et, blks in zip(ets, DIAG):
                              for db, off, w in blks:
                                  j = nf + db
                                  nc.tensor.matmul(
                                      cps[hi][:, S - w:S],
                                      vp[j][:, h * 65:h * 65 + 65],
                                      et[:, off:off + w],
                                      start=(j == 0), stop=(db == 3))
                      # normalize both heads
                      for hi in range(2):
                          rec = rp.tile([1, S], F32, tag="rec", name="rec")
                          nc.vector.reciprocal(rec[:], cps[hi][64:65, :])
                          rbc = rp.tile([64, S], F32, tag="rbc", name="rbc")
                          nc.gpsimd.partition_broadcast(rbc[:], rec[:])
                          nc.vector.tensor_mul(
                              ctxt[c][hi * 64:hi * 64 + 64, q_sl],
                              cps[hi][0:64, :], rbc[:])

          # =============== output projection ===============
          with tc.tile_pool(name="wop", bufs=1) as wop, \
               tc.tile_pool(name="op", bufs=3) as op:
              bo_f = wop.tile([1, DOUT], F32)
              nc.sync.dma_start(bo_f[:], bo_d[:])
              bob = wop.tile([128, DOUT], F32)
              nc.gpsimd.partition_broadcast(bob[:], bo_f[:])
              wo_t = []
              for c in range(CC):
                  t = wop.tile([128, DOUT], F32R, name=f"wot{c}")
                  nc.sync.dma_start(t[:], wo_d[c * 128:(c + 1) * 128, :])
                  wo_t.append(t)
              for m in range(NT):
                  for n in range(2):
                      po = pp.tile([128, 512], F32, tag="s", name="po")
                      for c in range(CC):
                          nc.tensor.matmul(
                              po[:], ctxt[c][:, m * 128:(m + 1) * 128],
                              wo_t[c][:, n * 512:(n + 1) * 512],
                              start=(c == 0), stop=(c == CC - 1))
                      ot = op.tile([128, 512], F32, tag="ot", name="ot")
                      nc.vector.tensor_add(ot[:], po[:],
                                           bob[:, n * 512:(n + 1) * 512])
                      nc.sync.dma_start(
                          out_d[m * 128:(m + 1) * 128, n * 512:(n + 1) * 512],
                          ot[:])

    nc.compile()
    return nc


_NC = None


def _get_nc():
    global _NC
    if _NC is None:
        _NC = _build_nc()
    return _NC


def make_in_maps(x, w_q, b_q, w_k, b_k, w_v, b_v, w_o, b_o):
    tri = np.triu(np.ones((128, 128), dtype=np.float32))
    in_maps = []
    for core in range(8):
        b, g = core // 2, core % 2
        sl = slice(g * DL, (g + 1) * DL)
        bo = b_o if g == 0 else np.zeros_like(b_o)
        in_maps.append({
            "xT": np.ascontiguousarray(x[b].T).astype(np.float32),
            "wq": np.ascontiguousarray(w_q[:, sl]).astype(np.float32),
            "wk": np.ascontiguousarray(w_k[:, sl]).astype(np.float32),
            "wv": np.ascontiguousarray(w_v[:, sl]).astype(np.float32),
            "bq": np.ascontiguousarray(b_q[sl].reshape(DL, 1)).astype(np.float32),
            "bk": np.ascontiguousarray(b_k[sl].reshape(DL, 1)).astype(np.float32),
            "bv": np.ascontiguousarray(b_v[sl].reshape(1, DL)).astype(np.float32),
            "wo": np.ascontiguousarray(w_o[sl, :]).astype(np.float32),
            "bo": np.ascontiguousarray(bo.reshape(1, DOUT)).astype(np.float32),
            "tri": tri,
        })
    return in_maps


def kernel(x, w_q, b_q, w_k, b_k, w_v, b_v, w_o, b_o):
    from concourse.bass_utils import run_bass_kernel_spmd
    nc = _get_nc()
    in_maps = make_in_maps(np.asarray(x), np.asarray(w_q), np.asarray(b_q),
                           np.asarray(w_k), np.asarray(b_k), np.asarray(w_v),
                           np.asarray(b_v), np.asarray(w_o), np.asarray(b_o))
    res = run_bass_kernel_spmd(nc, in_maps, core_ids=list(range(8)))
    outs = [r["out"] for r in res.results]
    full = np.stack([outs[2 * b] + outs[2 * b + 1] for b in range(B)])
    return full.astype(np.float32)
